# revision 18
# baseline (speedup 1.0000x reference)
"""Trainium2 Bass kernel for nn_GCEdecoder (sparse_attention), fp8 edition.

Reference computation (B=128, T=512, D=400, V=1024, A=128):
  vals = C_vals[:,0,:]                               # [V, D]
  S[b,v,t]  = sum_d H[b,t,d] * vals[v,d]             # scores
  P         = softmax over t (masked t < len_b)
  y_utts[b,v] = sum_d (sum_t P[b,v,t] H[b,t,d]) * W[d] + b0
  s2[b,a]   = sum_d C_acts[b,a,d] * c_utt[b,d]
  p2        = softmax_a(s2);  q[b,d] = sum_a p2 C_acts[b,a,d]
  y_acts[b,v] = sum_d q[b,d] vals[v,d]

Restructure (same as the f32r baseline): y_utts[b,v] =
(sum_t E[t,v]*hwm[b,t]) / (sum_t E[t,v]*m[b,t]) with E = exp(S - U_b),
hwm = (H@W + b0)*mask, m = mask.  Masking skips dead 128-wide t-chunks;
batches are sorted/dealt across cores and each slot's partial tail chunk is
packed into a shared merged-tail stream.

NEW: the dominant S matmuls run as fp8e4 (e4m3) DoubleRow matmuls at 0.5
cycles/row with 256 contraction rows per instruction.  S is computed exactly
enough via a hi+lo decomposition: H ~ Hh + Hl, vals ~ Vh + Vl (each e4m3),
S ~ Hh.Vh + Hl.Vh + Hh.Vl (the Hl.Vl term is dropped; measured y_utts rel
err ~5.5e-3 at tolerance 2e-2).  The three terms concatenate along the
contraction dim into 1200 rows (+80 pad) = 10 k-tiles = 5 DoubleRow passes
per 512-wide output half: 1280 PE cycles per (chunk, half) vs 2048 for the
f32r 4-pass version (37.5% less PE time).

Each chunk's two 512-wide halves accumulate into one 2-bank [128,1024] PSUM
tile read by a single 1024-wide exp (ACT is the near-co-bottleneck; the
1024-wide form saves one 185ns fixed access cost per chunk).  E and the
scoring columns are bf16 (cheaper y-flush matmuls); the y_acts epilogue runs
off separate bf16 vals/q tiles (fp8 vals would be too coarse there).  The
merged-tail flushes now store raw per-chunk num/den contributions straight
from PSUM to DRAM; the host adds them into y_utts (shorter device tail).

DMA layouts keep every transfer's innermost contiguous run >= 512B (the cost
model halves DMA bandwidth below that): ht is [128p, t, 10kt] fp8 with t
sliced to the live chunks, vt is [128p, 10kt, v] sliced along v.

Sharding: data-parallel over B across 8 cores (16 batches/core).
"""

import os
import time

import numpy as np
import ml_dtypes

import concourse.bacc as bacc
import concourse.mybir as mybir
import concourse.tile as tile
from concourse.bass_utils import run_bass_kernel_spmd

B, T, D, V, A = 128, 512, 400, 1024, 128
NCORES = 8
BPC = B // NCORES  # batches per core
NVC = V // 128  # 128-wide v chunks
NKT = 10  # fp8 concat k-tiles: [Hh(400); Hl(400); Hh(400)] + 80 pad
D1 = D + 1  # ca/cu padded with a fused-shift column
GRP = 4  # batches per s2/p2 activation group
NG = BPC // GRP
F32 = mybir.dt.float32
F32R = mybir.dt.float32r
BF16 = mybir.dt.bfloat16
F8 = mybir.dt.float8e4
DR = mybir.MatmulPerfMode.DoubleRow
EXP = mybir.ActivationFunctionType.Exp
E4M3 = ml_dtypes.float8_e4m3
NPBF16 = ml_dtypes.bfloat16

_cache = {}

HT_BUFS = int(os.environ.get("HT_BUFS", "3"))
E_BUFS = int(os.environ.get("E_BUFS", "7"))
PSS_BUFS = int(os.environ.get("PSS_BUFS", "3"))
PSY_BUFS = int(os.environ.get("PSY_BUFS", "1"))
CA_BUFS = int(os.environ.get("CA_BUFS", "9"))
EPI_JT = int(os.environ.get("EPI_JT", "1"))
FLUSH_LAG = int(os.environ.get("FLUSH_LAG", "3"))
CA_SYNC_N = int(os.environ.get("CA_SYNC_N", "0"))
VY_B = int(os.environ.get("VY_B", "8"))
MT_GRP = int(os.environ.get("MT_GRP", "3"))  # merged chunks per htt DMA group
WARM_N = int(os.environ.get("WARM_N", "25"))
CA_PRE = int(os.environ.get("CA_PRE", "2"))  # ca/cb prefetch depth in batches
FRONT_LAG = int(os.environ.get("FRONT_LAG", "2"))


def _mt_groups(M):
    """Split M merged chunks into DMA groups of MT_GRP."""
    sizes = []
    left = M
    while left > 0:
        sizes.append(min(MT_GRP, left))
        left -= MT_GRP
    return sizes


def build_program(chunk_counts, merge_meta):
    """chunk_counts[b] = number of FULL 128-wide t-chunks for this slot;
    merge_meta[m] = tuple of slots contributing to merged tail chunk m."""
    nc = bacc.Bacc("TRN2", target_bir_lowering=False, debug=False)
    M = len(merge_meta)
    smw = sum(2 * len(mm) for mm in merge_meta)
    smoff = np.concatenate([[0], np.cumsum([2 * len(mm) for mm in merge_meta])])
    gsizes = _mt_groups(M)

    # Per-core inputs (host pre-swizzled; see _prep_inputs below).
    ht = nc.dram_tensor("ht", (BPC, 128, T, NKT), F8, kind="ExternalInput")
    htts = [
        nc.dram_tensor(f"htt{g}", (128, 128 * gs, NKT), F8, kind="ExternalInput")
        for g, gs in enumerate(gsizes)
    ]
    vt = nc.dram_tensor("vt", (128, NKT, V), F8, kind="ExternalInput")
    vy = nc.dram_tensor("vy", (128, 4, V), BF16, kind="ExternalInput")
    smt = nc.dram_tensor("smt", (128, BPC, 4, 2), BF16, kind="ExternalInput")
    smm = nc.dram_tensor("smm", (128, max(smw, 2)), BF16, kind="ExternalInput")
    shf = nc.dram_tensor("shf", (128, BPC + M), F32, kind="ExternalInput")
    ca = nc.dram_tensor("ca", (BPC, A, D1), F32, kind="ExternalInput")
    cu = nc.dram_tensor("cu", (BPC, D1), F32R, kind="ExternalInput")
    yu = nc.dram_tensor("yu", (128, BPC, 2 * NVC), F32, kind="ExternalOutput")
    yms = [
        nc.dram_tensor(
            f"ym{m}", (128, NVC, 2 * len(merge_meta[m])), F32, kind="ExternalOutput"
        )
        for m in range(M)
    ]
    ya = nc.dram_tensor("ya", (128, NVC, BPC), F32, kind="ExternalOutput")
    d2o = nc.dram_tensor("d2o", (GRP, NG), F32, kind="ExternalOutput")

    with tile.TileContext(nc) as tc:
        with (
            tc.tile_pool(name="const", bufs=1) as cpool,
            tc.tile_pool(name="work", bufs=HT_BUFS) as wpool,
            tc.tile_pool(name="cain", bufs=CA_BUFS) as capool,
            tc.tile_pool(name="etile", bufs=E_BUFS) as epool,
            tc.tile_pool(name="psS", bufs=PSS_BUFS, space="PSUM") as psS,
            tc.tile_pool(name="psY", bufs=PSY_BUFS, space="PSUM") as psY,
            tc.tile_pool(name="psQ", bufs=1, space="PSUM") as psQ,
        ):
            # ---- constants / persistent tiles ----
            vt_sb = cpool.tile([128, NKT, V], F8)
            vy_sb = cpool.tile([128, 4, V], BF16)
            sm_sb = cpool.tile([128, BPC, 4, 2], BF16)
            smm_sb = cpool.tile([128, max(smw, 2)], BF16)
            bias_sb = cpool.tile([128, BPC + M], F32)
            onecol_sb = cpool.tile([128, 1], F32)
            nc.vector.memset(onecol_sb[:], 1.0)
            ones_sb = cpool.tile([1, 128], F32R)
            nc.vector.memset(ones_sb[:], 1.0)
            cu_sb = cpool.tile([1, BPC, D1], F32R)
            s2_all = cpool.tile([128, BPC], F32)
            # q^T accumulator across batches (bf16 for the bf16 ya matmuls)
            qt_sb = cpool.tile([128, 4, BPC], BF16)
            d2_sb = cpool.tile([GRP, NG], F32)
            yu_sb = cpool.tile([128, BPC, 2 * NVC], F32)
            yacts_sb = cpool.tile([128, NVC, BPC], F32)

            pend = []
            y_tiles = {}
            htm_tiles = []

            def _epilogue():
                # y_acts raw: out[v-part, b] = sum_d vals[v, d] q[b, d] as
                # bf16 matmuls (the /d2 division happens on the host).
                for cp in range(NVC // 2):
                    ya_ps = psQ.tile([128, 2, BPC], F32, tag="q")
                    for half in range(2):
                        c = 2 * cp + half
                        for j in range(4):
                            kp = 128 if j < 3 else 16
                            nc.tensor.matmul(
                                ya_ps[:, half, :],
                                vy_sb[0:kp, j, 128 * c : 128 * (c + 1)],
                                qt_sb[0:kp, j, :],
                                start=(j == 0),
                                stop=(j == 3),
                            )
                    nc.vector.tensor_copy(
                        yacts_sb[:, 2 * cp : 2 * cp + 2, :], ya_ps[:]
                    )
                nc.sync.dma_start(ya[:], yacts_sb[:])
                nc.sync.dma_start(d2o[:], d2_sb[:])

            def _flush_y(item):
                # single-shot bf16 matmuls into a per-chunk [128, 16] slice;
                # per-batch jt slices fold on the DVE at batch end (at most
                # one PSUM operand per DVE instruction)
                e_sb, bb, jt, cn = item
                y_ps = y_tiles[bb]
                for c in range(NVC):
                    nc.tensor.matmul(
                        y_ps[:, jt, 2 * c : 2 * c + 2],
                        e_sb[:, 128 * c : 128 * (c + 1)],
                        sm_sb[:, bb, jt, :],
                        start=True,
                        stop=True,
                    )
                if cn == 1:
                    nc.vector.tensor_copy(yu_sb[:, bb, :], y_ps[:, 0, :])
                elif jt == cn - 1:
                    acc = epool.tile([128, 2 * NVC], F32, tag="acc", name=f"acc_{bb}")
                    nc.vector.tensor_copy(acc[:], y_ps[:, 0, :])
                    for k in range(1, cn):
                        dst = acc[:] if k < cn - 1 else yu_sb[:, bb, :]
                        nc.vector.tensor_tensor(
                            dst, acc[:], y_ps[:, k, :], mybir.AluOpType.add
                        )

            def _s_chain(s_out, lhs_ap, vlo, vhi):
                """5 DoubleRow passes accumulating lhs^T @ vt[:, :, vlo:vhi]."""
                for k in range(5):
                    nc.tensor.matmul(
                        s_out,
                        lhs_ap(k),
                        vt_sb[:, 2 * k : 2 * k + 2, vlo:vhi],
                        start=(k == 0),
                        stop=(k == 4),
                        perf_mode=DR,
                    )

            def _ht_lhs(ht_sb, jt):
                def lhs_ap(k):
                    return ht_sb[
                        :, 128 * jt : 128 * (jt + 1), 2 * k : 2 * k + 2
                    ].transpose([0, 2, 1])

                return lhs_ap

            ca_keep = {}

            def _s2_batch(bb):
                # broadcast cu[bb] across partitions with one tiny f32r
                # matmul (no per-batch broadcast DMAs: the Pool SWDGE queue
                # sustains only ~1 DMA per 2us and serialized the old cb
                # stream), then a single fused DVE mult+reduce makes
                # s2_all[:, bb] (the -shift rides in cu's extra column)
                cb_ps = psQ.tile([128, D1], F32, tag="q", name=f"cb_ps_{bb}")
                nc.tensor.matmul(
                    cb_ps[:], ones_sb[:], cu_sb[0:1, bb, :],
                    start=True, stop=True,
                )
                scr = epool.tile([128, D1], F32, tag="scr")
                nc.vector.tensor_tensor_reduce(
                    scr[:],
                    ca_keep[bb][:],
                    cb_ps[:],
                    1.0,
                    0.0,
                    mybir.AluOpType.mult,
                    mybir.AluOpType.add,
                    s2_all[:, bb : bb + 1],
                )

            def _front_group(g):
                # p2 for GRP batches with one grouped exp, then the pooled
                # q^T and d2 columns
                p2g = epool.tile([128, GRP], F32, tag="p2")
                nc.scalar.activation(p2g[:], s2_all[:, GRP * g : GRP * (g + 1)], EXP)
                for i in range(GRP):
                    bb = GRP * g + i
                    ca_sb = ca_keep.pop(bb)
                    qt_ps = psQ.tile([128, 4], F32, tag="q")
                    for j in range(4):
                        mp = 128 if j < 3 else D - 384
                        nc.tensor.matmul(
                            qt_ps[0:mp, j : j + 1],
                            ca_sb[:, 128 * j : 128 * j + mp],
                            p2g[:, i : i + 1],
                            start=True,
                            stop=True,
                        )
                    nc.vector.tensor_copy(qt_sb[:, :, bb], qt_ps[:, 0:4])
                d2_ps = psQ.tile([GRP, 1], F32, tag="q")
                nc.tensor.matmul(
                    d2_ps[:], p2g[:], onecol_sb[:], start=True, stop=True
                )
                nc.vector.tensor_copy(d2_sb[:, g : g + 1], d2_ps[:])

            def _load_ca(bb):
                # ca on the scalar queue (the sync queue's trigger budget is
                # spent on ht/htt/stores)
                ca_sb = capool.tile([128, D1], F32, tag="ca")
                nc.scalar.dma_start(ca_sb[:], ca[bb])
                ca_keep[bb] = ca_sb

            if WARM_N:
                # dummy matmuls bridge the startup DMA wait: they hold the
                # PE's p-state ramp so the first real chunks run full-speed
                warm_sb = cpool.tile([128, 128], F32R)
                nc.vector.memset(warm_sb[:], 0.0)
                warm_ps = psS.tile([128, V], F32, tag="s", name="warm_ps")
                for _ in range(WARM_N):
                    nc.tensor.matmul(
                        warm_ps[:, 0:128],
                        warm_sb[:],
                        warm_sb[:],
                        start=True,
                        stop=True,
                    )

            for b in range(BPC):
                cn = chunk_counts[b]
                w = 128 * cn
                # ---- load this batch (only the live t columns) ----
                ht_sb = wpool.tile([128, T, NKT], F8, tag="ht")
                if cn > 0:
                    if b == 0:
                        for jt in range(cn):
                            nc.sync.dma_start(
                                ht_sb[:, 128 * jt : 128 * (jt + 1), :],
                                ht[b, :, 128 * jt : 128 * (jt + 1), :],
                            )
                    else:
                        nc.sync.dma_start(ht_sb[:, 0:w, :], ht[b, :, 0:w, :])
                else:
                    nc.vector.memset(yu_sb[:, b, :], 0.0)
                if b == BPC - 1:
                    # queue the merged-tail loads now
                    for g, gs in enumerate(gsizes):
                        htm_sb = wpool.tile(
                            [128, 128 * gs, NKT], F8, tag="htm", name=f"htm_{g}"
                        )
                        nc.sync.dma_start(htm_sb[:], htts[g][:])
                        for mi in range(gs):
                            htm_tiles.append((htm_sb, mi))
                if b == 0:
                    # vt split along v across the scalar and gpsimd queues;
                    # nothing else rides these queues until vt is done
                    for q in range(2):
                        nc.scalar.dma_start(
                            vt_sb[:, :, 256 * q : 256 * (q + 1)],
                            vt[:, :, 256 * q : 256 * (q + 1)],
                        )
                    for q in range(2, 4):
                        nc.gpsimd.dma_start(
                            vt_sb[:, :, 256 * q : 256 * (q + 1)],
                            vt[:, :, 256 * q : 256 * (q + 1)],
                        )
                    nc.scalar.dma_start(cu_sb[:], cu[:])
                    nc.scalar.dma_start(bias_sb[:], shf[:])
                    nc.scalar.dma_start(sm_sb[:], smt[:])
                    for pb in range(CA_PRE + 1):
                        _load_ca(pb)
                elif b + CA_PRE < BPC:
                    _load_ca(b + CA_PRE)
                if b == VY_B:
                    nc.gpsimd.dma_start(vy_sb[:], vy[:])
                if b == VY_B + 1:
                    nc.gpsimd.dma_start(smm_sb[:], smm[:])

                # per-batch s2 column (tiny PE broadcast + one DVE op);
                # for b0 it waits until after the chunks so the cu_sb wait
                # can't park the PE queue head during startup
                if b > 0:
                    _s2_batch(b)

                # ---- scores + exp; num/den matmuls lag behind ----
                if cn > 0:
                    y_tiles[b] = psY.tile(
                        [128, 4, 2 * NVC], F32, tag="y", name=f"y_ps_{b}"
                    )
                if b == 0:
                    # b0 in 256-wide quarter chains, per chunk, so each chain
                    # starts as soon as its ht chunk piece + vt v-piece land
                    for jt in range(cn):
                        s_ps = psS.tile([128, V], F32, tag="s", name=f"s0_{jt}")
                        lhs = _ht_lhs(ht_sb, jt)
                        for q in range(4):
                            _s_chain(
                                s_ps[:, 256 * q : 256 * (q + 1)],
                                lhs,
                                256 * q,
                                256 * (q + 1),
                            )
                        e_sb = epool.tile([128, V], BF16, tag="e")
                        nc.scalar.activation(
                            e_sb[:], s_ps[:], EXP, bias=bias_sb[:, b : b + 1]
                        )
                        pend.append((e_sb, b, jt, cn))
                        if len(pend) > FLUSH_LAG:
                            _flush_y(pend.pop(0))
                    _s2_batch(0)
                else:
                    for jt in range(cn):
                        s_ps = psS.tile([128, V], F32, tag="s")
                        lhs = _ht_lhs(ht_sb, jt)
                        for vc in range(2):
                            _s_chain(
                                s_ps[:, 512 * vc : 512 * (vc + 1)],
                                lhs,
                                512 * vc,
                                512 * (vc + 1),
                            )
                        e_sb = epool.tile([128, V], BF16, tag="e")
                        nc.scalar.activation(
                            e_sb[:], s_ps[:], EXP, bias=bias_sb[:, b : b + 1]
                        )
                        pend.append((e_sb, b, jt, cn))
                        if len(pend) > FLUSH_LAG:
                            _flush_y(pend.pop(0))
                        if b == BPC - 1 and jt == min(EPI_JT, cn - 1):
                            # last batch: the final group's front must precede
                            # the epilogue (it fills qt_sb/d2_sb for batches
                            # 12..15); both overlap the remaining S chunks
                            _front_group(NG - 1)
                            _epilogue()

                # fronts run FRONT_LAG batches after their group completes so
                # their exp/qt chains never park ahead of chunk work on the
                # in-order ACT and PE queues
                gdone = (b - FRONT_LAG) // GRP
                if (
                    b >= FRONT_LAG
                    and (b - FRONT_LAG) % GRP == GRP - 1
                    and 0 <= gdone < NG - 1
                ):
                    _front_group(gdone)
                if b == BPC - 1 and cn == 0:
                    _front_group(NG - 1)
                    _epilogue()

            while pend:
                _flush_y(pend.pop(0))

            # the folded per-batch num/den ship now; merged-tail raw
            # contributions go to DRAM separately and the host adds them
            nc.sync.dma_start(yu[:], yu_sb[:])

            # ---- merged tail chunks ----
            pendm = []

            def _flush_m(item):
                em_sb, m = item
                nb = len(merge_meta[m])
                off = int(smoff[m])
                ym_ps = psY.tile(
                    [128, NVC, 2 * nb], F32, tag="y", name=f"ym_ps_{m}"
                )
                for c in range(NVC):
                    nc.tensor.matmul(
                        ym_ps[:, c, :],
                        em_sb[:, 128 * c : 128 * (c + 1)],
                        smm_sb[:, off : off + 2 * nb],
                        start=True,
                        stop=True,
                    )
                ym_sb = epool.tile([128, NVC, 2 * nb], F32, tag="acc", name=f"ym_{m}")
                nc.vector.tensor_copy(ym_sb[:], ym_ps[:])
                nc.sync.dma_start(yms[m][:], ym_sb[:])

            for m in range(M):
                htm_sb, mi = htm_tiles[m]
                em_sb = epool.tile([128, V], BF16, tag="e", name=f"em_{m}")
                lhs = _ht_lhs(htm_sb, mi)
                sm_ps = psS.tile([128, V], F32, tag="s", name=f"sm_ps_{m}")
                nparts = 4 if m == M - 1 else 2
                wv = V // nparts
                for vc in range(nparts):
                    _s_chain(
                        sm_ps[:, wv * vc : wv * (vc + 1)], lhs, wv * vc, wv * (vc + 1)
                    )
                    if nparts == 4:
                        # flush per quarter on the critical tail
                        nc.scalar.activation(
                            em_sb[:, wv * vc : wv * (vc + 1)],
                            sm_ps[:, wv * vc : wv * (vc + 1)],
                            EXP,
                            bias=bias_sb[:, BPC + m : BPC + m + 1],
                        )
                if nparts == 2:
                    nc.scalar.activation(
                        em_sb[:],
                        sm_ps[:],
                        EXP,
                        bias=bias_sb[:, BPC + m : BPC + m + 1],
                    )
                pendm.append((em_sb, m))
                if len(pendm) > 1:
                    _flush_m(pendm.pop(0))

            while pendm:
                _flush_m(pendm.pop(0))

    nc.compile()
    return nc


def _plan_slots(lens):
    """Sort batches by len (desc) and deal them across cores so each SPMD
    slot holds near-equal lens.  Each slot's last (partial) t-chunk is
    diverted into a shared "merged tail" stream packed 128 t-rows at a time
    across slots.

    Returns (order, chunk_counts, merge_meta, stream)."""
    lens = np.asarray(lens).astype(np.int64)
    order = np.argsort(-lens, kind="stable")
    slot_lens = lens[order].reshape(BPC, NCORES)
    slot_max = slot_lens.max(axis=1)
    cn = np.clip(np.ceil(slot_max / 128).astype(int), 1, 4)
    full = cn - 1
    segs = []
    for j in range(BPC):
        t0 = 128 * int(full[j])
        wseg = int(slot_max[j]) - t0
        if wseg > 0:
            segs.append((wseg, j, t0))
    # ascending width, largest last: the final merged chunk then holds a
    # single slot so the tail store is tiny.  Pad-align the largest segment
    # to a chunk boundary when that doesn't cost an extra chunk.
    segs.sort()
    total = sum(wseg for wseg, _j, _t0 in segs)
    M0 = (total + 127) // 128
    pad = 0
    if segs:
        head = total - segs[-1][0]
        pad = (-head) % 128
        if (head + pad + segs[-1][0] + 127) // 128 > M0:
            pad = 0
    stream = []
    for i, (wseg, j, t0) in enumerate(segs):
        if i == len(segs) - 1 and pad:
            stream.extend((-1, 0) for _ in range(pad))
        stream.extend((j, t) for t in range(t0, t0 + wseg))
    M = (len(stream) + 127) // 128
    stream.extend((-1, 0) for _ in range(128 * M - len(stream)))
    merge_meta = []
    for m in range(M):
        seen = []
        for s, _t in stream[128 * m : 128 * (m + 1)]:
            if s >= 0 and s not in seen:
                seen.append(s)
        merge_meta.append(tuple(seen))
    return (
        order,
        tuple(int(c) for c in full),
        tuple(merge_meta),
        tuple(stream),
    )


def _q8(x):
    return np.asarray(x, np.float32).astype(E4M3)


def _prep_inputs(H_utt, c_utt, C_acts, C_vals, W_score, b_score, utterance_len):
    """Host-side quantization + swizzling into the kernel's per-core layouts."""
    H_utt = np.ascontiguousarray(H_utt, dtype=np.float32)
    c_utt = np.asarray(c_utt, dtype=np.float32)
    C_acts = np.ascontiguousarray(C_acts, dtype=np.float32)
    vals = np.asarray(C_vals, dtype=np.float32)[:, 0, :]  # [V, D]
    W = np.asarray(W_score, dtype=np.float32)[0]  # [D]
    b0 = np.float32(np.asarray(b_score, dtype=np.float32).reshape(-1)[0])
    lens = np.asarray(utterance_len).astype(np.int64)

    order, chunk_counts, merge_meta, stream = _plan_slots(lens)
    M = len(merge_meta)
    smw = max(sum(2 * len(mm) for mm in merge_meta), 2)
    gsizes = _mt_groups(M)
    stream_slot = np.array([s for s, _t in stream], np.int64).reshape(M, 128)
    stream_t = np.array([t for _s, t in stream], np.int64).reshape(M, 128)

    # fp8 hi+lo concat: rows [Hh(400); Hl(400); Hh(400); pad80] along d
    HT = H_utt.transpose(0, 2, 1)  # [B, D, T]
    Hh8 = _q8(HT)
    Hl8 = _q8(HT - Hh8.astype(np.float32))
    hcat = np.zeros((B, 128 * NKT, T), E4M3)
    hcat[:, 0:D] = Hh8
    hcat[:, 400:800] = Hl8
    hcat[:, 800:1200] = Hh8
    # ht_all[b, p, t, kt] = hcat[b, 128*kt + p, t]
    ht_all = np.ascontiguousarray(
        hcat.reshape(B, NKT, 128, T).transpose(0, 2, 3, 1)
    )

    # vt concat rows: [Vh; Vh; Vl; pad] matching the lhs cat
    VT = vals.T  # [D, V]
    Vh8 = _q8(VT)
    Vl8 = _q8(VT - Vh8.astype(np.float32))
    vcat = np.zeros((128 * NKT, V), E4M3)
    vcat[0:D] = Vh8
    vcat[400:800] = Vh8
    vcat[800:1200] = Vl8
    vt_host = np.ascontiguousarray(vcat.reshape(NKT, 128, V).transpose(1, 0, 2))

    # bf16 vals for the y_acts epilogue: vy[p, j, v] = vals[v, 128j+p]
    vtp = np.zeros((512, V), np.float32)
    vtp[:D] = VT
    vy_host = np.ascontiguousarray(
        vtp.reshape(4, 128, V).transpose(1, 0, 2).astype(NPBF16)
    )

    # scoring columns [B, T, 2] = (hw*mask, mask) in bf16, laid out
    # [128, B, 4, 2]
    hw = H_utt.reshape(B * T, D) @ W
    hw = hw.reshape(B, T) + b0
    mask = (np.arange(T)[None, :] < lens[:, None]).astype(np.float32)
    sm = np.empty((B, T, 2), np.float32)
    sm[:, :, 0] = hw * mask
    sm[:, :, 1] = mask
    sm_host = np.ascontiguousarray(
        sm.reshape(B, 4, 128, 2).transpose(2, 0, 1, 3).astype(NPBF16)
    )

    # Per-batch exp shifts (exact after normalization).  y_utts: 0.85x a
    # strided-sample max of the true scores.  y_acts: exact row max, folded
    # into cu's extra column.
    s_samp = np.einsum(
        "btd,vd->btv", H_utt[:, ::8, :].astype(np.float64), vals[::8].astype(np.float64)
    )
    shift_u = np.maximum(0.85 * s_samp.max(axis=(1, 2)), 1.0)  # [B]
    s2_full = np.einsum(
        "bad,bd->ba", C_acts.astype(np.float64), c_utt.astype(np.float64)
    )
    shift_a = s2_full.max(axis=1).astype(np.float32)  # [B]
    shift_u = shift_u.astype(np.float32)

    in_maps = []
    for c in range(NCORES):
        sel = order[c::NCORES]  # slot j -> original batch index

        smm_core = np.zeros((128, smw), np.float32)
        shf_core = np.zeros((128, BPC + M), np.float32)
        shf_core[:, :BPC] = -shift_u[sel][None, :]
        htt_cores = [
            np.zeros((128, 128 * gs, NKT), E4M3) for gs in gsizes
        ]
        off = 0
        for m in range(M):
            sl = stream_slot[m]
            ts = stream_t[m]
            valid = sl >= 0
            bsel = sel[np.clip(sl, 0, BPC - 1)]
            # gather: [s(128), p(128), kt] -> [p, s, kt]
            gat = ht_all[bsel, :, ts, :].astype(np.float32)
            gat *= valid[:, None, None]
            g, mi = divmod(m, MT_GRP)
            htt_cores[g][:, 128 * mi : 128 * (mi + 1), :] = gat.transpose(
                1, 0, 2
            ).astype(E4M3)
            shf_core[:, BPC + m] = np.where(valid, -shift_u[bsel], 0.0)
            for k, sj in enumerate(merge_meta[m]):
                rows = valid & (sl == sj)
                bj = sel[sj]
                smm_core[rows, off + 2 * k] = (hw[bj] * mask[bj])[ts[rows]]
                smm_core[rows, off + 2 * k + 1] = mask[bj][ts[rows]]
            off += 2 * len(merge_meta[m])

        ca_core = np.empty((BPC, A, D1), np.float32)
        ca_core[:, :, :D] = C_acts[sel]
        ca_core[:, :, D] = 1.0
        cu_core = np.empty((BPC, D1), np.float32)
        cu_core[:, :D] = c_utt[sel]
        cu_core[:, D] = -shift_a[sel]

        im = {
            "ht": np.ascontiguousarray(ht_all[sel]),
            "vt": vt_host,
            "vy": vy_host,
            "smt": np.ascontiguousarray(sm_host[:, sel]),
            "smm": smm_core.astype(NPBF16),
            "shf": shf_core,
            "ca": ca_core,
            "cu": cu_core,
        }
        for g in range(len(gsizes)):
            im[f"htt{g}"] = htt_cores[g]
        in_maps.append(im)
    return in_maps, order, chunk_counts, merge_meta


def _gather_outputs(res, order, merge_meta):
    """Scatter per-core slot outputs back to original batch order, folding
    the merged-tail raw contributions in on the host."""
    M = len(merge_meta)
    y_utts = np.empty((B, V), np.float32)
    y_acts = np.empty((B, V), np.float32)
    for c in range(NCORES):
        sel = order[c::NCORES]
        r = res.results[c]
        # yu: [128(p), BPC(slot), (num, den) x NVC]
        yu = np.asarray(r["yu"]).astype(np.float64).transpose(1, 2, 0)
        num = np.ascontiguousarray(yu[:, 0::2, :]).reshape(BPC, V)
        den = np.ascontiguousarray(yu[:, 1::2, :]).reshape(BPC, V)
        for m in range(M):
            ym = np.asarray(r[f"ym{m}"]).astype(np.float64)  # [128, NVC, 2nb]
            for k, sj in enumerate(merge_meta[m]):
                num[sj] += ym[:, :, 2 * k].T.reshape(V)
                den[sj] += ym[:, :, 2 * k + 1].T.reshape(V)
        y_utts[sel] = (num / den).astype(np.float32)
        # ya: [128(p), NVC(chunk), BPC(slot)] raw; divide by d2 = sum_a p2
        ya_raw = np.asarray(r["ya"]).transpose(2, 1, 0).reshape(BPC, V)
        d2 = np.asarray(r["d2o"]).T.reshape(BPC)  # [GRP, NG] -> slot 4g+i
        y_acts[sel] = ya_raw / d2[:, None]
    return y_utts, y_acts


def _get_program(chunk_counts, merge_meta):
    key = ("nc", chunk_counts, merge_meta)
    if key not in _cache:
        _cache[key] = build_program(chunk_counts, merge_meta)
    _cache["nc"] = _cache[key]  # latest program, for test harness TimelineSim
    return _cache[key]


def _reset_jax_backend():
    """Tear down the PJRT/axon client so the next call reconnects."""
    try:
        import jax
        from jax._src import xla_bridge

        jax.clear_caches()
        xla_bridge._clear_backends()
    except Exception:  # noqa: BLE001 - best effort
        pass


def _run_with_retry(nc, in_maps, attempts=4, trace=False):
    """First execution of a freshly compiled NEFF occasionally dies with
    NRT_EXEC_UNIT_UNRECOVERABLE on this deployment; reconnect and retry."""
    last = None
    for i in range(attempts):
        try:
            return run_bass_kernel_spmd(
                nc, in_maps, core_ids=list(range(NCORES)), trace=trace
            )
        except Exception as e:  # noqa: BLE001 - any runtime/transport error
            last = e
            time.sleep(2.0 * (i + 1))
            _reset_jax_backend()
    raise last


def kernel(H_utt, c_utt, C_acts, C_vals, W_score, b_score, utterance_len, **_):
    in_maps, order, chunk_counts, merge_meta = _prep_inputs(
        H_utt, c_utt, C_acts, C_vals, W_score, b_score, utterance_len
    )
    nc = _get_program(chunk_counts, merge_meta)
    res = _run_with_retry(nc, in_maps)
    return _gather_outputs(res, order, merge_meta)


def kernel_traced(trace=True, **inputs):
    """Like kernel() but returns (outputs, BassKernelResults) with profiling."""
    in_maps, order, chunk_counts, merge_meta = _prep_inputs(
        **{
            k: inputs[k]
            for k in (
                "H_utt",
                "c_utt",
                "C_acts",
                "C_vals",
                "W_score",
                "b_score",
                "utterance_len",
            )
        }
    )
    nc = _get_program(chunk_counts, merge_meta)
    res = _run_with_retry(nc, in_maps, trace=trace)
    return _gather_outputs(res, order, merge_meta), res


if __name__ == "__main__":
    rng = np.random.default_rng(0)
    inputs = {
        "H_utt": rng.standard_normal((B, T, D), dtype=np.float32),
        "c_utt": rng.standard_normal((B, D), dtype=np.float32),
        "C_acts": rng.standard_normal((B, A, D), dtype=np.float32),
        "C_vals": rng.standard_normal((V, 1, D), dtype=np.float32),
        "W_score": rng.standard_normal((1, D), dtype=np.float32) / np.sqrt(D),
        "b_score": np.zeros((1,), np.float32),
        "utterance_len": rng.integers(T // 2, T + 1, size=(B,)).astype(np.int64),
    }
    y_utts, y_acts = kernel(**inputs)
    print("y_utts", y_utts.shape, "y_acts", y_acts.shape)


# revision 20
# speedup vs baseline: 1.0622x; 1.0622x over previous
"""Trainium2 Bass kernel for nn_GCEdecoder (sparse_attention), fp8 edition.

Reference computation (B=128, T=512, D=400, V=1024, A=128):
  vals = C_vals[:,0,:]                               # [V, D]
  S[b,v,t]  = sum_d H[b,t,d] * vals[v,d]             # scores
  P         = softmax over t (masked t < len_b)
  y_utts[b,v] = sum_d (sum_t P[b,v,t] H[b,t,d]) * W[d] + b0
  s2[b,a]   = sum_d C_acts[b,a,d] * c_utt[b,d]
  p2        = softmax_a(s2);  q[b,d] = sum_a p2 C_acts[b,a,d]
  y_acts[b,v] = sum_d q[b,d] vals[v,d]

Restructure (same as the f32r baseline): y_utts[b,v] =
(sum_t E[t,v]*hwm[b,t]) / (sum_t E[t,v]*m[b,t]) with E = exp(S - U_b),
hwm = (H@W + b0)*mask, m = mask.  Masking skips dead 128-wide t-chunks;
batches are sorted/dealt across cores and each slot's partial tail chunk is
packed into a shared merged-tail stream.

NEW: the dominant S matmuls run as fp8e4 (e4m3) DoubleRow matmuls at 0.5
cycles/row with 256 contraction rows per instruction.  S is computed exactly
enough via a hi+lo decomposition: H ~ Hh + Hl, vals ~ Vh + Vl (each e4m3),
S ~ Hh.Vh + Hl.Vh + Hh.Vl (the Hl.Vl term is dropped; measured y_utts rel
err ~5.5e-3 at tolerance 2e-2).  The three terms concatenate along the
contraction dim into 1200 rows (+80 pad) = 10 k-tiles = 5 DoubleRow passes
per 512-wide output half: 1280 PE cycles per (chunk, half) vs 2048 for the
f32r 4-pass version (37.5% less PE time).

Each chunk's two 512-wide halves accumulate into one 2-bank [128,1024] PSUM
tile read by a single 1024-wide exp (ACT is the near-co-bottleneck; the
1024-wide form saves one 185ns fixed access cost per chunk).  E and the
scoring columns are bf16 (cheaper y-flush matmuls); the y_acts epilogue runs
off separate bf16 vals/q tiles (fp8 vals would be too coarse there).  The
merged-tail flushes now store raw per-chunk num/den contributions straight
from PSUM to DRAM; the host adds them into y_utts (shorter device tail).

DMA layouts keep every transfer's innermost contiguous run >= 512B (the cost
model halves DMA bandwidth below that): ht is [128p, t, 10kt] fp8 with t
sliced to the live chunks, vt is [128p, 10kt, v] sliced along v.

Sharding: data-parallel over B across 8 cores (16 batches/core).
"""

import os
import time

import numpy as np
import ml_dtypes

import concourse.bacc as bacc
import concourse.mybir as mybir
import concourse.tile as tile
from concourse.bass_utils import run_bass_kernel_spmd

B, T, D, V, A = 128, 512, 400, 1024, 128
NCORES = 8
BPC = B // NCORES  # batches per core
NVC = V // 128  # 128-wide v chunks
NKT = 10  # fp8 concat k-tiles: [Hh(400); Hl(400); Hh(400)] + 80 pad
D1 = D + 1  # ca/cu padded with a fused-shift column
GRP = 4  # batches per s2/p2 activation group
NG = BPC // GRP
F32 = mybir.dt.float32
F32R = mybir.dt.float32r
BF16 = mybir.dt.bfloat16
F8 = mybir.dt.float8e4
DR = mybir.MatmulPerfMode.DoubleRow
EXP = mybir.ActivationFunctionType.Exp
E4M3 = ml_dtypes.float8_e4m3
NPBF16 = ml_dtypes.bfloat16

_cache = {}

HT_BUFS = int(os.environ.get("HT_BUFS", "3"))
E_BUFS = int(os.environ.get("E_BUFS", "7"))
PSS_BUFS = int(os.environ.get("PSS_BUFS", "3"))
PSY_BUFS = int(os.environ.get("PSY_BUFS", "1"))
CA_BUFS = int(os.environ.get("CA_BUFS", "9"))
EPI_JT = int(os.environ.get("EPI_JT", "1"))
FLUSH_LAG = int(os.environ.get("FLUSH_LAG", "3"))
CA_SYNC_N = int(os.environ.get("CA_SYNC_N", "0"))
VY_B = int(os.environ.get("VY_B", "12"))
MT_GRP = int(os.environ.get("MT_GRP", "3"))  # merged chunks per htt DMA group
WARM_N = int(os.environ.get("WARM_N", "25"))
CA_PRE = int(os.environ.get("CA_PRE", "2"))  # ca/cb prefetch depth in batches
FRONT_LAG = int(os.environ.get("FRONT_LAG", "2"))


def _mt_groups(M):
    """Split M merged chunks into DMA groups of MT_GRP."""
    sizes = []
    left = M
    while left > 0:
        sizes.append(min(MT_GRP, left))
        left -= MT_GRP
    return sizes


def build_program(chunk_counts, merge_meta):
    """chunk_counts[b] = number of FULL 128-wide t-chunks for this slot;
    merge_meta[m] = tuple of slots contributing to merged tail chunk m."""
    nc = bacc.Bacc("TRN2", target_bir_lowering=False, debug=False)
    M = len(merge_meta)
    smw = sum(2 * len(mm) for mm in merge_meta)
    smoff = np.concatenate([[0], np.cumsum([2 * len(mm) for mm in merge_meta])])
    gsizes = _mt_groups(M)

    # Per-core inputs (host pre-swizzled; see _prep_inputs below).
    ht = nc.dram_tensor("ht", (BPC, 128, T, NKT), F8, kind="ExternalInput")
    htts = [
        nc.dram_tensor(f"htt{g}", (128, 128 * gs, NKT), F8, kind="ExternalInput")
        for g, gs in enumerate(gsizes)
    ]
    vt = nc.dram_tensor("vt", (128, NKT, V), F8, kind="ExternalInput")
    vy = nc.dram_tensor("vy", (128, 4, V), BF16, kind="ExternalInput")
    smt = nc.dram_tensor("smt", (128, BPC, 4, 2), BF16, kind="ExternalInput")
    smm = nc.dram_tensor("smm", (128, max(smw, 2)), BF16, kind="ExternalInput")
    shf = nc.dram_tensor("shf", (128, BPC + M), F32, kind="ExternalInput")
    ca = nc.dram_tensor("ca", (BPC, A, D1), F32, kind="ExternalInput")
    cu = nc.dram_tensor("cu", (BPC, D1), F32R, kind="ExternalInput")
    yu = nc.dram_tensor("yu", (128, BPC, 2 * NVC), F32, kind="ExternalOutput")
    yms = [
        nc.dram_tensor(
            f"ym{m}", (128, NVC, 2 * len(merge_meta[m])), F32, kind="ExternalOutput"
        )
        for m in range(M)
    ]
    ya = nc.dram_tensor("ya", (128, NVC, BPC), F32, kind="ExternalOutput")
    d2o = nc.dram_tensor("d2o", (GRP, NG), F32, kind="ExternalOutput")

    with tile.TileContext(nc) as tc:
        with (
            tc.tile_pool(name="const", bufs=1) as cpool,
            tc.tile_pool(name="work", bufs=HT_BUFS) as wpool,
            tc.tile_pool(name="cain", bufs=CA_BUFS) as capool,
            tc.tile_pool(name="etile", bufs=E_BUFS) as epool,
            tc.tile_pool(name="psS", bufs=PSS_BUFS, space="PSUM") as psS,
            tc.tile_pool(name="psY", bufs=PSY_BUFS, space="PSUM") as psY,
            tc.tile_pool(name="psQ", bufs=1, space="PSUM") as psQ,
        ):
            # ---- constants / persistent tiles ----
            vt_sb = cpool.tile([128, NKT, V], F8)
            vy_sb = cpool.tile([128, 4, V], BF16)
            sm_sb = cpool.tile([128, BPC, 4, 2], BF16)
            smm_sb = cpool.tile([128, max(smw, 2)], BF16)
            bias_sb = cpool.tile([128, BPC + M], F32)
            onecol_sb = cpool.tile([128, 1], F32)
            nc.vector.memset(onecol_sb[:], 1.0)
            ones_sb = cpool.tile([1, 128], F32R)
            nc.vector.memset(ones_sb[:], 1.0)
            cu_sb = cpool.tile([1, BPC, D1], F32R)
            s2_all = cpool.tile([128, BPC], F32)
            # q^T accumulator across batches (bf16 for the bf16 ya matmuls)
            qt_sb = cpool.tile([128, 4, BPC], BF16)
            d2_sb = cpool.tile([GRP, NG], F32)
            yu_sb = cpool.tile([128, BPC, 2 * NVC], F32)
            yacts_sb = cpool.tile([128, NVC, BPC], F32)

            pend = []
            y_tiles = {}
            htm_tiles = []

            def _epilogue():
                # y_acts raw: out[v-part, b] = sum_d vals[v, d] q[b, d] as
                # bf16 matmuls (the /d2 division happens on the host).
                for cp in range(NVC // 2):
                    ya_ps = psQ.tile([128, 2, BPC], F32, tag="q")
                    for half in range(2):
                        c = 2 * cp + half
                        for j in range(4):
                            kp = 128 if j < 3 else 16
                            nc.tensor.matmul(
                                ya_ps[:, half, :],
                                vy_sb[0:kp, j, 128 * c : 128 * (c + 1)],
                                qt_sb[0:kp, j, :],
                                start=(j == 0),
                                stop=(j == 3),
                            )
                    nc.vector.tensor_copy(
                        yacts_sb[:, 2 * cp : 2 * cp + 2, :], ya_ps[:]
                    )
                nc.sync.dma_start(ya[:], yacts_sb[:])
                nc.sync.dma_start(d2o[:], d2_sb[:])

            def _flush_y(item):
                # single-shot bf16 matmuls into a per-chunk [128, 16] slice;
                # per-batch jt slices fold on the DVE at batch end (at most
                # one PSUM operand per DVE instruction)
                e_sb, bb, jt, cn = item
                y_ps = y_tiles[bb]
                for c in range(NVC):
                    nc.tensor.matmul(
                        y_ps[:, jt, 2 * c : 2 * c + 2],
                        e_sb[:, 128 * c : 128 * (c + 1)],
                        sm_sb[:, bb, jt, :],
                        start=True,
                        stop=True,
                    )
                if cn == 1:
                    nc.vector.tensor_copy(yu_sb[:, bb, :], y_ps[:, 0, :])
                elif jt == cn - 1:
                    acc = epool.tile([128, 2 * NVC], F32, tag="acc", name=f"acc_{bb}")
                    nc.vector.tensor_copy(acc[:], y_ps[:, 0, :])
                    for k in range(1, cn):
                        dst = acc[:] if k < cn - 1 else yu_sb[:, bb, :]
                        nc.vector.tensor_tensor(
                            dst, acc[:], y_ps[:, k, :], mybir.AluOpType.add
                        )

            def _s_chain(s_out, lhs_ap, vlo, vhi):
                """5 DoubleRow passes accumulating lhs^T @ vt[:, :, vlo:vhi]."""
                for k in range(5):
                    nc.tensor.matmul(
                        s_out,
                        lhs_ap(k),
                        vt_sb[:, 2 * k : 2 * k + 2, vlo:vhi],
                        start=(k == 0),
                        stop=(k == 4),
                        perf_mode=DR,
                    )

            def _ht_lhs(ht_sb, jt):
                def lhs_ap(k):
                    return ht_sb[
                        :, 128 * jt : 128 * (jt + 1), 2 * k : 2 * k + 2
                    ].transpose([0, 2, 1])

                return lhs_ap

            ca_keep = {}

            def _s2_batch(bb):
                # broadcast cu[bb] across partitions with one tiny f32r
                # matmul (no per-batch broadcast DMAs: the Pool SWDGE queue
                # sustains only ~1 DMA per 2us and serialized the old cb
                # stream), then a single fused DVE mult+reduce makes
                # s2_all[:, bb] (the -shift rides in cu's extra column)
                cb_ps = psQ.tile([128, D1], F32, tag="q", name=f"cb_ps_{bb}")
                nc.tensor.matmul(
                    cb_ps[:], ones_sb[:], cu_sb[0:1, bb, :],
                    start=True, stop=True,
                )
                scr = epool.tile([128, D1], F32, tag="scr")
                nc.vector.tensor_tensor_reduce(
                    scr[:],
                    ca_keep[bb][:],
                    cb_ps[:],
                    1.0,
                    0.0,
                    mybir.AluOpType.mult,
                    mybir.AluOpType.add,
                    s2_all[:, bb : bb + 1],
                )

            def _front_group(g):
                # p2 for GRP batches with one grouped exp, then the pooled
                # q^T and d2 columns
                p2g = epool.tile([128, GRP], F32, tag="p2")
                nc.scalar.activation(p2g[:], s2_all[:, GRP * g : GRP * (g + 1)], EXP)
                for i in range(GRP):
                    bb = GRP * g + i
                    ca_sb = ca_keep.pop(bb)
                    qt_ps = psQ.tile([128, 4], F32, tag="q")
                    for j in range(4):
                        mp = 128 if j < 3 else D - 384
                        nc.tensor.matmul(
                            qt_ps[0:mp, j : j + 1],
                            ca_sb[:, 128 * j : 128 * j + mp],
                            p2g[:, i : i + 1],
                            start=True,
                            stop=True,
                        )
                    nc.vector.tensor_copy(qt_sb[:, :, bb], qt_ps[:, 0:4])
                d2_ps = psQ.tile([GRP, 1], F32, tag="q")
                nc.tensor.matmul(
                    d2_ps[:], p2g[:], onecol_sb[:], start=True, stop=True
                )
                nc.vector.tensor_copy(d2_sb[:, g : g + 1], d2_ps[:])

            def _load_ca(bb):
                # ca on the scalar queue (the sync queue's trigger budget is
                # spent on ht/htt/stores)
                ca_sb = capool.tile([128, D1], F32, tag="ca")
                nc.scalar.dma_start(ca_sb[:], ca[bb])
                ca_keep[bb] = ca_sb

            if WARM_N:
                # dummy matmuls bridge the startup DMA wait: they hold the
                # PE's p-state ramp so the first real chunks run full-speed
                warm_sb = cpool.tile([128, 128], F32R)
                nc.vector.memset(warm_sb[:], 0.0)
                warm_ps = psS.tile([128, V], F32, tag="s", name="warm_ps")
                for _ in range(WARM_N):
                    nc.tensor.matmul(
                        warm_ps[:, 0:128],
                        warm_sb[:],
                        warm_sb[:],
                        start=True,
                        stop=True,
                    )

            for b in range(BPC):
                cn = chunk_counts[b]
                w = 128 * cn
                # ---- load this batch (only the live t columns) ----
                ht_sb = wpool.tile([128, T, NKT], F8, tag="ht")
                if cn > 0:
                    nc.sync.dma_start(ht_sb[:, 0:w, :], ht[b, :, 0:w, :])
                else:
                    nc.vector.memset(yu_sb[:, b, :], 0.0)
                if b == BPC - 1:
                    # queue the merged-tail loads now
                    for g, gs in enumerate(gsizes):
                        htm_sb = wpool.tile(
                            [128, 128 * gs, NKT], F8, tag="htm", name=f"htm_{g}"
                        )
                        nc.sync.dma_start(htm_sb[:], htts[g][:])
                        for mi in range(gs):
                            htm_tiles.append((htm_sb, mi))
                if b == 0:
                    # vt in 3 v-pieces across the scalar and gpsimd queues;
                    # startup DMA count is kept low: concurrent DMAs are
                    # round-robined onto ~8 serial HW rings, and a burst
                    # parks small control loads behind 1.4us ht transfers
                    for q in range(2):
                        nc.scalar.dma_start(
                            vt_sb[:, :, 256 * q : 256 * (q + 1)],
                            vt[:, :, 256 * q : 256 * (q + 1)],
                        )
                    nc.gpsimd.dma_start(vt_sb[:, :, 512:1024], vt[:, :, 512:1024])
                    nc.scalar.dma_start(cu_sb[:], cu[:])
                    nc.scalar.dma_start(bias_sb[:], shf[:])
                    nc.scalar.dma_start(sm_sb[:], smt[:])
                elif b == 1:
                    for pb in range(3):
                        _load_ca(pb)
                elif b + 1 < BPC:
                    _load_ca(b + 1)
                if b == VY_B:
                    nc.gpsimd.dma_start(vy_sb[:], vy[:])
                if b == VY_B + 1:
                    nc.gpsimd.dma_start(smm_sb[:], smm[:])

                # per-batch s2 columns (tiny PE broadcast + one DVE op),
                # one batch behind the ca loads; the last batch emits its own
                # column early so the in-chunk front/epilogue can read it
                if b > 0:
                    _s2_batch(b - 1)
                if b == BPC - 1:
                    _s2_batch(b)

                # ---- scores + exp; num/den matmuls lag behind ----
                if cn > 0:
                    y_tiles[b] = psY.tile(
                        [128, 4, 2 * NVC], F32, tag="y", name=f"y_ps_{b}"
                    )
                if b == 0:
                    # b0 in 256-wide quarter chains, per chunk, so each chain
                    # starts as soon as its ht chunk piece + vt v-piece land
                    for jt in range(cn):
                        s_ps = psS.tile([128, V], F32, tag="s", name=f"s0_{jt}")
                        lhs = _ht_lhs(ht_sb, jt)
                        for q in range(4):
                            _s_chain(
                                s_ps[:, 256 * q : 256 * (q + 1)],
                                lhs,
                                256 * q,
                                256 * (q + 1),
                            )
                        e_sb = epool.tile([128, V], BF16, tag="e")
                        nc.scalar.activation(
                            e_sb[:], s_ps[:], EXP, bias=bias_sb[:, b : b + 1]
                        )
                        pend.append((e_sb, b, jt, cn))
                        if len(pend) > FLUSH_LAG:
                            _flush_y(pend.pop(0))
                else:
                    for jt in range(cn):
                        s_ps = psS.tile([128, V], F32, tag="s")
                        lhs = _ht_lhs(ht_sb, jt)
                        for vc in range(2):
                            _s_chain(
                                s_ps[:, 512 * vc : 512 * (vc + 1)],
                                lhs,
                                512 * vc,
                                512 * (vc + 1),
                            )
                        e_sb = epool.tile([128, V], BF16, tag="e")
                        nc.scalar.activation(
                            e_sb[:], s_ps[:], EXP, bias=bias_sb[:, b : b + 1]
                        )
                        pend.append((e_sb, b, jt, cn))
                        if len(pend) > FLUSH_LAG:
                            _flush_y(pend.pop(0))
                        if b == BPC - 1 and jt == min(EPI_JT, cn - 1):
                            # last batch: the final group's front must precede
                            # the epilogue (it fills qt_sb/d2_sb for batches
                            # 12..15); both overlap the remaining S chunks
                            _front_group(NG - 1)
                            _epilogue()

                # fronts run FRONT_LAG batches after their group completes so
                # their exp/qt chains never park ahead of chunk work on the
                # in-order ACT and PE queues
                gdone = (b - FRONT_LAG) // GRP
                if (
                    b >= FRONT_LAG
                    and (b - FRONT_LAG) % GRP == GRP - 1
                    and 0 <= gdone < NG - 1
                ):
                    _front_group(gdone)
                if b == BPC - 1 and cn == 0:
                    _front_group(NG - 1)
                    _epilogue()

            while pend:
                _flush_y(pend.pop(0))

            # the folded per-batch num/den ship now; merged-tail raw
            # contributions go to DRAM separately and the host adds them
            nc.sync.dma_start(yu[:], yu_sb[:])

            # ---- merged tail chunks ----
            pendm = []

            def _flush_m(item):
                em_sb, m = item
                nb = len(merge_meta[m])
                off = int(smoff[m])
                ym_ps = psY.tile(
                    [128, NVC, 2 * nb], F32, tag="y", name=f"ym_ps_{m}"
                )
                for c in range(NVC):
                    nc.tensor.matmul(
                        ym_ps[:, c, :],
                        em_sb[:, 128 * c : 128 * (c + 1)],
                        smm_sb[:, off : off + 2 * nb],
                        start=True,
                        stop=True,
                    )
                ym_sb = epool.tile([128, NVC, 2 * nb], F32, tag="acc", name=f"ym_{m}")
                nc.vector.tensor_copy(ym_sb[:], ym_ps[:])
                nc.sync.dma_start(yms[m][:], ym_sb[:])

            for m in range(M):
                htm_sb, mi = htm_tiles[m]
                em_sb = epool.tile([128, V], BF16, tag="e", name=f"em_{m}")
                lhs = _ht_lhs(htm_sb, mi)
                sm_ps = psS.tile([128, V], F32, tag="s", name=f"sm_ps_{m}")
                nparts = 4 if m == M - 1 else 2
                wv = V // nparts
                for vc in range(nparts):
                    _s_chain(
                        sm_ps[:, wv * vc : wv * (vc + 1)], lhs, wv * vc, wv * (vc + 1)
                    )
                    if nparts == 4:
                        # flush per quarter on the critical tail
                        nc.scalar.activation(
                            em_sb[:, wv * vc : wv * (vc + 1)],
                            sm_ps[:, wv * vc : wv * (vc + 1)],
                            EXP,
                            bias=bias_sb[:, BPC + m : BPC + m + 1],
                        )
                if nparts == 2:
                    nc.scalar.activation(
                        em_sb[:],
                        sm_ps[:],
                        EXP,
                        bias=bias_sb[:, BPC + m : BPC + m + 1],
                    )
                pendm.append((em_sb, m))
                if len(pendm) > 1:
                    _flush_m(pendm.pop(0))

            while pendm:
                _flush_m(pendm.pop(0))

    nc.compile()
    return nc


def _plan_slots(lens):
    """Sort batches by len (desc) and deal them across cores so each SPMD
    slot holds near-equal lens.  Each slot's last (partial) t-chunk is
    diverted into a shared "merged tail" stream packed 128 t-rows at a time
    across slots.

    Returns (order, chunk_counts, merge_meta, stream)."""
    lens = np.asarray(lens).astype(np.int64)
    order = np.argsort(-lens, kind="stable")
    slot_lens = lens[order].reshape(BPC, NCORES)
    slot_max = slot_lens.max(axis=1)
    cn = np.clip(np.ceil(slot_max / 128).astype(int), 1, 4)
    full = cn - 1
    segs = []
    for j in range(BPC):
        t0 = 128 * int(full[j])
        wseg = int(slot_max[j]) - t0
        if wseg > 0:
            segs.append((wseg, j, t0))
    # ascending width, largest last: the final merged chunk then holds a
    # single slot so the tail store is tiny.  Pad-align the largest segment
    # to a chunk boundary when that doesn't cost an extra chunk.
    segs.sort()
    total = sum(wseg for wseg, _j, _t0 in segs)
    M0 = (total + 127) // 128
    pad = 0
    if segs:
        head = total - segs[-1][0]
        pad = (-head) % 128
        if (head + pad + segs[-1][0] + 127) // 128 > M0:
            pad = 0
    stream = []
    for i, (wseg, j, t0) in enumerate(segs):
        if i == len(segs) - 1 and pad:
            stream.extend((-1, 0) for _ in range(pad))
        stream.extend((j, t) for t in range(t0, t0 + wseg))
    M = (len(stream) + 127) // 128
    stream.extend((-1, 0) for _ in range(128 * M - len(stream)))
    merge_meta = []
    for m in range(M):
        seen = []
        for s, _t in stream[128 * m : 128 * (m + 1)]:
            if s >= 0 and s not in seen:
                seen.append(s)
        merge_meta.append(tuple(seen))
    return (
        order,
        tuple(int(c) for c in full),
        tuple(merge_meta),
        tuple(stream),
    )


def _q8(x):
    return np.asarray(x, np.float32).astype(E4M3)


def _prep_inputs(H_utt, c_utt, C_acts, C_vals, W_score, b_score, utterance_len):
    """Host-side quantization + swizzling into the kernel's per-core layouts."""
    H_utt = np.ascontiguousarray(H_utt, dtype=np.float32)
    c_utt = np.asarray(c_utt, dtype=np.float32)
    C_acts = np.ascontiguousarray(C_acts, dtype=np.float32)
    vals = np.asarray(C_vals, dtype=np.float32)[:, 0, :]  # [V, D]
    W = np.asarray(W_score, dtype=np.float32)[0]  # [D]
    b0 = np.float32(np.asarray(b_score, dtype=np.float32).reshape(-1)[0])
    lens = np.asarray(utterance_len).astype(np.int64)

    order, chunk_counts, merge_meta, stream = _plan_slots(lens)
    M = len(merge_meta)
    smw = max(sum(2 * len(mm) for mm in merge_meta), 2)
    gsizes = _mt_groups(M)
    stream_slot = np.array([s for s, _t in stream], np.int64).reshape(M, 128)
    stream_t = np.array([t for _s, t in stream], np.int64).reshape(M, 128)

    # fp8 hi+lo concat: rows [Hh(400); Hl(400); Hh(400); pad80] along d
    HT = H_utt.transpose(0, 2, 1)  # [B, D, T]
    Hh8 = _q8(HT)
    Hl8 = _q8(HT - Hh8.astype(np.float32))
    hcat = np.zeros((B, 128 * NKT, T), E4M3)
    hcat[:, 0:D] = Hh8
    hcat[:, 400:800] = Hl8
    hcat[:, 800:1200] = Hh8
    # ht_all[b, p, t, kt] = hcat[b, 128*kt + p, t]
    ht_all = np.ascontiguousarray(
        hcat.reshape(B, NKT, 128, T).transpose(0, 2, 3, 1)
    )

    # vt concat rows: [Vh; Vh; Vl; pad] matching the lhs cat
    VT = vals.T  # [D, V]
    Vh8 = _q8(VT)
    Vl8 = _q8(VT - Vh8.astype(np.float32))
    vcat = np.zeros((128 * NKT, V), E4M3)
    vcat[0:D] = Vh8
    vcat[400:800] = Vh8
    vcat[800:1200] = Vl8
    vt_host = np.ascontiguousarray(vcat.reshape(NKT, 128, V).transpose(1, 0, 2))

    # bf16 vals for the y_acts epilogue: vy[p, j, v] = vals[v, 128j+p]
    vtp = np.zeros((512, V), np.float32)
    vtp[:D] = VT
    vy_host = np.ascontiguousarray(
        vtp.reshape(4, 128, V).transpose(1, 0, 2).astype(NPBF16)
    )

    # scoring columns [B, T, 2] = (hw*mask, mask) in bf16, laid out
    # [128, B, 4, 2]
    hw = H_utt.reshape(B * T, D) @ W
    hw = hw.reshape(B, T) + b0
    mask = (np.arange(T)[None, :] < lens[:, None]).astype(np.float32)
    sm = np.empty((B, T, 2), np.float32)
    sm[:, :, 0] = hw * mask
    sm[:, :, 1] = mask
    sm_host = np.ascontiguousarray(
        sm.reshape(B, 4, 128, 2).transpose(2, 0, 1, 3).astype(NPBF16)
    )

    # Per-batch exp shifts (exact after normalization).  y_utts: 0.85x a
    # strided-sample max of the true scores.  y_acts: exact row max, folded
    # into cu's extra column.
    s_samp = np.einsum(
        "btd,vd->btv", H_utt[:, ::8, :].astype(np.float64), vals[::8].astype(np.float64)
    )
    shift_u = np.maximum(0.85 * s_samp.max(axis=(1, 2)), 1.0)  # [B]
    s2_full = np.einsum(
        "bad,bd->ba", C_acts.astype(np.float64), c_utt.astype(np.float64)
    )
    shift_a = s2_full.max(axis=1).astype(np.float32)  # [B]
    shift_u = shift_u.astype(np.float32)

    in_maps = []
    for c in range(NCORES):
        sel = order[c::NCORES]  # slot j -> original batch index

        smm_core = np.zeros((128, smw), np.float32)
        shf_core = np.zeros((128, BPC + M), np.float32)
        shf_core[:, :BPC] = -shift_u[sel][None, :]
        htt_cores = [
            np.zeros((128, 128 * gs, NKT), E4M3) for gs in gsizes
        ]
        off = 0
        for m in range(M):
            sl = stream_slot[m]
            ts = stream_t[m]
            valid = sl >= 0
            bsel = sel[np.clip(sl, 0, BPC - 1)]
            # gather: [s(128), p(128), kt] -> [p, s, kt]
            gat = ht_all[bsel, :, ts, :].astype(np.float32)
            gat *= valid[:, None, None]
            g, mi = divmod(m, MT_GRP)
            htt_cores[g][:, 128 * mi : 128 * (mi + 1), :] = gat.transpose(
                1, 0, 2
            ).astype(E4M3)
            shf_core[:, BPC + m] = np.where(valid, -shift_u[bsel], 0.0)
            for k, sj in enumerate(merge_meta[m]):
                rows = valid & (sl == sj)
                bj = sel[sj]
                smm_core[rows, off + 2 * k] = (hw[bj] * mask[bj])[ts[rows]]
                smm_core[rows, off + 2 * k + 1] = mask[bj][ts[rows]]
            off += 2 * len(merge_meta[m])

        ca_core = np.empty((BPC, A, D1), np.float32)
        ca_core[:, :, :D] = C_acts[sel]
        ca_core[:, :, D] = 1.0
        cu_core = np.empty((BPC, D1), np.float32)
        cu_core[:, :D] = c_utt[sel]
        cu_core[:, D] = -shift_a[sel]

        im = {
            "ht": np.ascontiguousarray(ht_all[sel]),
            "vt": vt_host,
            "vy": vy_host,
            "smt": np.ascontiguousarray(sm_host[:, sel]),
            "smm": smm_core.astype(NPBF16),
            "shf": shf_core,
            "ca": ca_core,
            "cu": cu_core,
        }
        for g in range(len(gsizes)):
            im[f"htt{g}"] = htt_cores[g]
        in_maps.append(im)
    return in_maps, order, chunk_counts, merge_meta


def _gather_outputs(res, order, merge_meta):
    """Scatter per-core slot outputs back to original batch order, folding
    the merged-tail raw contributions in on the host."""
    M = len(merge_meta)
    y_utts = np.empty((B, V), np.float32)
    y_acts = np.empty((B, V), np.float32)
    for c in range(NCORES):
        sel = order[c::NCORES]
        r = res.results[c]
        # yu: [128(p), BPC(slot), (num, den) x NVC]
        yu = np.asarray(r["yu"]).astype(np.float64).transpose(1, 2, 0)
        num = np.ascontiguousarray(yu[:, 0::2, :]).reshape(BPC, V)
        den = np.ascontiguousarray(yu[:, 1::2, :]).reshape(BPC, V)
        for m in range(M):
            ym = np.asarray(r[f"ym{m}"]).astype(np.float64)  # [128, NVC, 2nb]
            for k, sj in enumerate(merge_meta[m]):
                num[sj] += ym[:, :, 2 * k].T.reshape(V)
                den[sj] += ym[:, :, 2 * k + 1].T.reshape(V)
        y_utts[sel] = (num / den).astype(np.float32)
        # ya: [128(p), NVC(chunk), BPC(slot)] raw; divide by d2 = sum_a p2
        ya_raw = np.asarray(r["ya"]).transpose(2, 1, 0).reshape(BPC, V)
        d2 = np.asarray(r["d2o"]).T.reshape(BPC)  # [GRP, NG] -> slot 4g+i
        y_acts[sel] = ya_raw / d2[:, None]
    return y_utts, y_acts


def _get_program(chunk_counts, merge_meta):
    key = ("nc", chunk_counts, merge_meta)
    if key not in _cache:
        _cache[key] = build_program(chunk_counts, merge_meta)
    _cache["nc"] = _cache[key]  # latest program, for test harness TimelineSim
    return _cache[key]


def _reset_jax_backend():
    """Tear down the PJRT/axon client so the next call reconnects."""
    try:
        import jax
        from jax._src import xla_bridge

        jax.clear_caches()
        xla_bridge._clear_backends()
    except Exception:  # noqa: BLE001 - best effort
        pass


def _run_with_retry(nc, in_maps, attempts=4, trace=False):
    """First execution of a freshly compiled NEFF occasionally dies with
    NRT_EXEC_UNIT_UNRECOVERABLE on this deployment; reconnect and retry."""
    last = None
    for i in range(attempts):
        try:
            return run_bass_kernel_spmd(
                nc, in_maps, core_ids=list(range(NCORES)), trace=trace
            )
        except Exception as e:  # noqa: BLE001 - any runtime/transport error
            last = e
            time.sleep(2.0 * (i + 1))
            _reset_jax_backend()
    raise last


def kernel(H_utt, c_utt, C_acts, C_vals, W_score, b_score, utterance_len, **_):
    in_maps, order, chunk_counts, merge_meta = _prep_inputs(
        H_utt, c_utt, C_acts, C_vals, W_score, b_score, utterance_len
    )
    nc = _get_program(chunk_counts, merge_meta)
    res = _run_with_retry(nc, in_maps)
    return _gather_outputs(res, order, merge_meta)


def kernel_traced(trace=True, **inputs):
    """Like kernel() but returns (outputs, BassKernelResults) with profiling."""
    in_maps, order, chunk_counts, merge_meta = _prep_inputs(
        **{
            k: inputs[k]
            for k in (
                "H_utt",
                "c_utt",
                "C_acts",
                "C_vals",
                "W_score",
                "b_score",
                "utterance_len",
            )
        }
    )
    nc = _get_program(chunk_counts, merge_meta)
    res = _run_with_retry(nc, in_maps, trace=trace)
    return _gather_outputs(res, order, merge_meta), res


if __name__ == "__main__":
    rng = np.random.default_rng(0)
    inputs = {
        "H_utt": rng.standard_normal((B, T, D), dtype=np.float32),
        "c_utt": rng.standard_normal((B, D), dtype=np.float32),
        "C_acts": rng.standard_normal((B, A, D), dtype=np.float32),
        "C_vals": rng.standard_normal((V, 1, D), dtype=np.float32),
        "W_score": rng.standard_normal((1, D), dtype=np.float32) / np.sqrt(D),
        "b_score": np.zeros((1,), np.float32),
        "utterance_len": rng.integers(T // 2, T + 1, size=(B,)).astype(np.int64),
    }
    y_utts, y_acts = kernel(**inputs)
    print("y_utts", y_utts.shape, "y_acts", y_acts.shape)
